# revision 1
# baseline (speedup 1.0000x reference)
"""Multi-head attention (B=4, N=2048, D=1024, H=16) on 8 TRN2 NeuronCores.

Sharding: DP=4 over batch x TP=2 over heads (megatron style).
  core c = 2*batch + j   (j in {0,1} = head-group half)
  - inputs:  x[batch] transposed -> xT [D, N]  (bf16)
  - W_qkv column-sharded: wq/wk/wv = W_qkv[:, {q,k,v} block, heads j*8:(j+1)*8]
  - W_proj row-sharded:   wp = W_proj[j*512:(j+1)*512, :]
  - per-core partial out [N, D]; host sums the TP pair (+ bias via per-core
    bias input that is b_proj on j==0 and zeros on j==1).

Per-core math (bf16 matmuls, fp32 PSUM accumulation):
  qT/kT  [feat, tok] = W^T @ xT          (feature-major, head pairs stacked)
  v      [tok, feat]                     (token-major), augmented with a
                                         ones-column per head (softmax sums)
  per head h, query-tile qh:
    S^T chunk [128 keys, QT] = kT_chunk.T @ qT     (K = hd = 64)
    A^T = exp(scale * S^T)  (ScalarE, PSUM->SBUF, bf16)
    outT_aug [65, QT] += v_aug_chunk.T @ A^T       (row 64 = softmax sums)
    normalize: one copy PSUM->SBUF frees the PSUM slot; sums row bounced
    via DRAM for a stride-0 partition-broadcast DMA, reciprocal_approx_fast,
    multiply -> att_outT [feat, tok] (feature-major == proj lhsT layout)
  proj: out[tok_block] = att_outT_chunk.T @ wp + bias
"""

import numpy as np

B, N, D, H = 4, 2048, 1024, 16
HD = 64
NCORES = 8
TP = 2
HLOC = H // TP          # 8 heads per core
FDIM = HLOC * HD        # 512

_PROG_CACHE = {}


def _build_program(tok, d, h_loc, hd, debug=False, repeat=1):
    """Build the single-core Bass/Tile program (same program runs SPMD on all cores)."""
    import concourse.tile as tile
    from concourse import bacc, mybir

    f32 = mybir.dt.float32
    bf16 = mybir.dt.bfloat16
    Exp = mybir.ActivationFunctionType.Exp

    P = 128
    DC = d // P                 # contraction chunks for QKV (8)
    NP = h_loc // 2             # head pairs (4)
    TB = tok // P               # token blocks (16)
    KC = tok // P               # key chunks (16)
    fdim = h_loc * hd           # local feature dim (512)
    FC = fdim // P              # proj contraction chunks (4)
    QT = min(tok, 1024)         # query tile
    QH = tok // QT              # query-halves (2)
    MMN = 512                   # max matmul free dim per instruction
    scale = float(hd) ** -0.5

    nc = bacc.Bacc("TRN2", target_bir_lowering=False, debug=debug)

    xT = nc.dram_tensor("xT", [d, tok], bf16, kind="ExternalInput")
    wq = nc.dram_tensor("wq", [d, fdim], bf16, kind="ExternalInput")
    wk = nc.dram_tensor("wk", [d, fdim], bf16, kind="ExternalInput")
    wv = nc.dram_tensor("wv", [d, fdim], bf16, kind="ExternalInput")
    wp = nc.dram_tensor("wp", [fdim, d], bf16, kind="ExternalInput")
    bias = nc.dram_tensor("bias", [P, d], f32, kind="ExternalInput")
    out = nc.dram_tensor("out", [tok, d], f32, kind="ExternalOutput")

    with tile.TileContext(nc) as tc:
        with (
            tc.tile_pool(name="sing", bufs=1) as sing,
            tc.tile_pool(name="psA", bufs=3, space="PSUM") as psA,
            tc.tile_pool(name="psO", bufs=1, space="PSUM") as psO,
            tc.tile_pool(name="work", bufs=4) as work,
            tc.tile_pool(name="atp", bufs=7) as atp,
            tc.tile_pool(name="outp", bufs=3) as outp,
            tc.tile_pool(name="dscr", bufs=8, space="DRAM") as dscr,
        ):
          for _rep in range(repeat):
            # ---- resident loads (xT chunked so compute starts ASAP) -----------
            wv_sb = sing.tile([P, DC, fdim], bf16)
            nc.gpsimd.dma_start(out=wv_sb, in_=wv[:, :].rearrange("(c p) m -> p c m", p=P))
            xT_sb = sing.tile([P, DC, tok], bf16)
            for c in range(DC):
                # alternate queues so the xT stream isn't bottlenecked on one
                eng = nc.sync if c % 2 == 0 else nc.scalar
                eng.dma_start(
                    out=xT_sb[:, c, :],
                    in_=xT[c * P:(c + 1) * P, :])
            wq_sb = sing.tile([P, DC, fdim], bf16)
            nc.gpsimd.dma_start(out=wq_sb, in_=wq[:, :].rearrange("(c p) m -> p c m", p=P))
            wk_sb = sing.tile([P, DC, fdim], bf16)
            nc.gpsimd.dma_start(out=wk_sb, in_=wk[:, :].rearrange("(c p) m -> p c m", p=P))
            wp_sb = sing.tile([P, FC, d], bf16)
            nc.gpsimd.dma_start(out=wp_sb, in_=wp[:, :].rearrange("(c p) m -> p c m", p=P))
            bias_sb = sing.tile([P, d], f32)
            nc.gpsimd.dma_start(out=bias_sb, in_=bias[:, :])

            qT_sb = sing.tile([P, NP, tok], bf16)
            kT_sb = sing.tile([P, NP, tok], bf16)
            vaug_sb = sing.tile([P, KC, h_loc, hd + 1], bf16)
            nc.vector.memset(vaug_sb, 1.0)  # ones column survives the v copies
            aoT_sb = sing.tile([P, NP, tok], bf16)

            # ---- v (token-major) into vaug -------------------------------------
            for tb in range(TB):
                ps = psA.tile([P, QT], f32, tag="ps")
                for c in range(DC):
                    for m0 in range(0, fdim, MMN):
                        ml = min(MMN, fdim - m0)
                        nc.tensor.matmul(
                            ps[:, m0:m0 + ml],
                            xT_sb[:, c, tb * P:(tb + 1) * P],
                            wv_sb[:, c, m0:m0 + ml],
                            start=(c == 0),
                            stop=(c == DC - 1),
                        )
                nc.vector.tensor_copy(
                    vaug_sb[:, tb, :, 0:hd],
                    ps[:, 0:fdim].rearrange("p (h e) -> p h e", h=h_loc),
                )

            # ---- q/k projections (all pairs up front) --------------------------
            for p in range(NP):
                for w_sb, dst in ((wq_sb, qT_sb), (wk_sb, kT_sb)):
                    for n0 in range(0, tok, QT):
                        ps = psA.tile([P, QT], f32, tag="ps")
                        for c in range(DC):
                            for m0 in range(0, QT, MMN):
                                ml = min(MMN, QT - m0)
                                nc.tensor.matmul(
                                    ps[:, m0:m0 + ml],
                                    w_sb[:, c, p * P:(p + 1) * P],
                                    xT_sb[:, c, n0 + m0:n0 + m0 + ml],
                                    start=(c == 0),
                                    stop=(c == DC - 1),
                                )
                        nc.vector.tensor_copy(dst[:, p, n0:n0 + QT], ps)

            # ---- attention (qh outer so proj can interleave) -------------------
            def proj_block(tb):
                pp = psA.tile([P, d], f32, tag="ps", name="pp")
                for fc in range(FC):
                    for m0 in range(0, d, MMN):
                        ml = min(MMN, d - m0)
                        nc.tensor.matmul(
                            pp[:, m0:m0 + ml],
                            aoT_sb[:, fc, tb * P:(tb + 1) * P],
                            wp_sb[:, fc, m0:m0 + ml],
                            start=(fc == 0),
                            stop=(fc == FC - 1),
                        )
                ot = outp.tile([P, d], f32, tag="ot", name="ot")
                nc.vector.tensor_add(ot, pp, bias_sb)
                oeng = nc.sync if tb % 2 == 0 else nc.scalar
                oeng.dma_start(out=out[tb * P:(tb + 1) * P, :], in_=ot)

            for qh in range(QH):
                q0 = qh * QT
                for p in range(NP):
                  for e in range(2):
                    po = psO.tile([hd + 1, QT], f32, tag="po", name="po")
                    for kc in range(KC):
                        ps = psA.tile([P, QT], f32, tag="ps")
                        for m0 in range(0, QT, MMN):
                            ml = min(MMN, QT - m0)
                            nc.tensor.matmul(
                                ps[:, m0:m0 + ml],
                                kT_sb[e * hd:(e + 1) * hd, p, kc * P:(kc + 1) * P],
                                qT_sb[e * hd:(e + 1) * hd, p, q0 + m0:q0 + m0 + ml],
                                start=True,
                                stop=True,
                            )
                        at = atp.tile([P, QT], bf16, tag="at")
                        nc.scalar.activation(at, ps, Exp, scale=scale)
                        for m0 in range(0, QT, MMN):
                            ml = min(MMN, QT - m0)
                            nc.tensor.matmul(
                                po[:, m0:m0 + ml],
                                vaug_sb[:, kc, 2 * p + e, :],
                                at[:, m0:m0 + ml],
                                start=(kc == 0),
                                stop=(kc == KC - 1),
                            )
                    if True:
                        # one fast copy frees the PSUM slot; the whole
                        # normalize chain then runs off the critical path
                        stg = work.tile([hd + 1, QT], f32, tag="stg")
                        nc.vector.tensor_copy(stg, po)
                        # bounce sums row via DRAM for a stride-0
                        # partition-broadcast read (DMA engines are idle)
                        sums_dr = dscr.tile([1, QT], f32, tag="sums_dr")
                        nc.sync.dma_start(out=sums_dr, in_=stg[hd:hd + 1, :])
                        rec_src = work.tile([hd, QT], f32, tag="rsrc")
                        nc.sync.dma_start(
                            out=rec_src,
                            in_=sums_dr.partition_broadcast(hd))
                        rec = work.tile([hd, QT], f32, tag="rec")
                        nc.vector.reciprocal_approx_fast(rec, rec_src)
                        nc.vector.tensor_mul(
                            aoT_sb[e * hd:(e + 1) * hd, p, q0:q0 + QT],
                            stg[0:hd, :],
                            rec,
                        )
            for tb in range(TB):
                proj_block(tb)

    nc.compile()
    return nc


def get_program(tok=N, d=D, h_loc=HLOC, hd=HD, debug=False, repeat=1):
    key = (tok, d, h_loc, hd, debug, repeat)
    if key not in _PROG_CACHE:
        _PROG_CACHE[key] = _build_program(tok, d, h_loc, hd, debug=debug,
                                          repeat=repeat)
    return _PROG_CACHE[key]


def make_in_maps(inputs_f32, W_qkv, W_proj, b_proj):
    """Shard full inputs into the 8 per-core input dicts."""
    import ml_dtypes

    bf16 = ml_dtypes.bfloat16
    in_maps = []
    for core in range(NCORES):
        b_idx, j = divmod(core, TP)
        f0, f1 = j * FDIM, (j + 1) * FDIM
        xT = np.ascontiguousarray(inputs_f32[b_idx].T).astype(bf16)
        wq_s = np.ascontiguousarray(W_qkv[:, f0:f1]).astype(bf16)
        wk_s = np.ascontiguousarray(W_qkv[:, D + f0:D + f1]).astype(bf16)
        wv_s = np.ascontiguousarray(W_qkv[:, 2 * D + f0:2 * D + f1]).astype(bf16)
        wp_s = np.ascontiguousarray(W_proj[f0:f1, :]).astype(bf16)
        if j == 0:
            bias_rep = np.broadcast_to(b_proj.astype(np.float32), (128, D)).copy()
        else:
            bias_rep = np.zeros((128, D), np.float32)
        in_maps.append(
            {"xT": xT, "wq": wq_s, "wk": wk_s, "wv": wv_s, "wp": wp_s,
             "bias": bias_rep}
        )
    return in_maps


def kernel(inputs, W_qkv, W_proj, b_proj):
    from concourse.bass_utils import run_bass_kernel_spmd

    inputs = np.asarray(inputs, dtype=np.float32)
    W_qkv = np.asarray(W_qkv, dtype=np.float32)
    W_proj = np.asarray(W_proj, dtype=np.float32)
    b_proj = np.asarray(b_proj, dtype=np.float32)

    nc = get_program()
    in_maps = make_in_maps(inputs, W_qkv, W_proj, b_proj)
    res = run_bass_kernel_spmd(nc, in_maps, core_ids=list(range(NCORES)))
    outs = [r["out"].astype(np.float32) for r in res.results]
    full = np.stack([outs[TP * b] + outs[TP * b + 1] for b in range(B)], axis=0)
    return full



# revision 6
# speedup vs baseline: 1.0015x; 1.0015x over previous
"""Multi-head attention (B=4, N=2048, D=1024, H=16) on 8 TRN2 NeuronCores.

Sharding: DP=4 over batch x TP=2 over heads (megatron style).
  core c = 2*batch + j   (j in {0,1} = head-group half)

Per-core pipeline (vs the bf16 baseline):
  - QKV projections run in fp8e4 (e4m3) with perf_mode=DoubleRow: the
    d-contraction is folded host-side into [128 part, DC2, 2, *] so each
    matmul contracts 256 d-rows (2 weights/PE cell) -> ~2x fewer PE cycles.
    W_qkv is pre-scaled by 2^6 so its values clear the e4m3 denormal range;
    the scaling cancels in softmax (q,k: via the exp scale; v: the
    ones-column of vaug is set to 2^6 so att/sums ratio is unchanged).
  - QK^T (K=64) issues the two head-halves e=0/e=1 back-to-back with
    tile_position (0,0)/(64,0): the 64x128-row-tiled matmuls execute
    concurrently in the two halves of the PE array -> ~2x on scores.
  - softmax exp alternates between ScalarE (exact activation) and VectorE
    (Schraudolph bit-trick: bf16 bits = trunc(S*A + B) computed by one
    tensor_scalar into a uint16 view) so the evacuation keeps pace with
    the PE instead of serializing behind one engine.
  - per-kc score pairs share one 2-bank PSUM tile [128, 2, 512]; one exp
    instruction covers both halves (amortizes the 352-cycle ACT overhead).
  - normalization: sums row bounced via DRAM for a partition-broadcast,
    reciprocal_approx_fast on DVE, final scale on GpSimd (Pool).
"""

import numpy as np

B, N, D, H = 4, 2048, 1024, 16
HD = 64
NCORES = 8
TP = 2
HLOC = H // TP          # 8 heads per core
FDIM = HLOC * HD        # 512

WS = 64.0               # 2^6 host-side scale on W_qkv (fp8 denormal escape)
USE_FP8_PROJ = False    # fp8e4+DoubleRow for the q/k/v projections (too lossy: ~3e-2)
DVE_EXP = True          # odd kc chunks use the DVE bit-trick exp
SCH_C = 5.6             # Schraudolph constant (min-max rel err ~3.3%)

_PROG_CACHE = {}


def _build_program(tok, d, h_loc, hd, debug=False, repeat=1):
    """Build the single-core Bass/Tile program (same program runs SPMD on all cores)."""
    import concourse.tile as tile
    from concourse import bacc, mybir

    f32 = mybir.dt.float32
    bf16 = mybir.dt.bfloat16
    f8 = mybir.dt.float8e4
    u16 = mybir.dt.uint16
    Exp = mybir.ActivationFunctionType.Exp
    Mult = mybir.AluOpType.mult
    Add = mybir.AluOpType.add
    DR = mybir.MatmulPerfMode.DoubleRow

    P = 128
    DC2 = d // 256              # folded contraction chunks (4): 256 d-rows each
    NP = h_loc // 2             # head pairs (4)
    TB = tok // P               # token blocks (16)
    KC = tok // P               # key chunks (16)
    fdim = h_loc * hd           # local feature dim (512)
    FC = fdim // P              # proj contraction chunks (4)
    QT = 512                    # query tile (1 PSUM bank)
    QH = tok // QT              # query tiles (4)
    MMN = 512
    scale_eff = float(hd) ** -0.5 / (WS * WS) if USE_FP8_PROJ else float(hd) ** -0.5
    log2e = 1.4426950408889634
    A_sch = 128.0 * log2e * scale_eff
    B_sch = 127.0 * 128.0 - SCH_C + 0.5

    wdt = f8 if USE_FP8_PROJ else bf16

    nc = bacc.Bacc("TRN2", target_bir_lowering=False, debug=debug)

    # host-folded layouts: [p, c2, i, *] with d_row = c2*256 + i*128 + p
    xT = nc.dram_tensor("xT", [P, DC2, 2, tok], wdt, kind="ExternalInput")
    wq = nc.dram_tensor("wq", [P, DC2, 2, fdim], wdt, kind="ExternalInput")
    wk = nc.dram_tensor("wk", [P, DC2, 2, fdim], wdt, kind="ExternalInput")
    wv = nc.dram_tensor("wv", [P, DC2, 2, fdim], wdt, kind="ExternalInput")
    wp = nc.dram_tensor("wp", [P, FC, d], bf16, kind="ExternalInput")
    bias = nc.dram_tensor("bias", [P, d], f32, kind="ExternalInput")
    out = nc.dram_tensor("out", [tok, d], f32, kind="ExternalOutput")

    with tile.TileContext(nc) as tc:
        with (
            tc.tile_pool(name="sing", bufs=1) as sing,
            tc.tile_pool(name="psA", bufs=3, space="PSUM") as psA,
            tc.tile_pool(name="psO", bufs=1, space="PSUM") as psO,
            tc.tile_pool(name="atp", bufs=4) as atp,
            tc.tile_pool(name="work", bufs=2) as work,
            tc.tile_pool(name="outp", bufs=3) as outp,
            tc.tile_pool(name="dscr", bufs=8, space="DRAM") as dscr,
        ):
          for _rep in range(repeat):
            # ---- resident loads ------------------------------------------------
            wv_sb = sing.tile([P, DC2, 2, fdim], wdt)
            nc.gpsimd.dma_start(out=wv_sb, in_=wv[:, :, :, :])
            xT_sb = sing.tile([P, DC2, 2, tok], wdt)
            for c in range(DC2):
                eng = nc.sync if c % 2 == 0 else nc.gpsimd
                eng.dma_start(out=xT_sb[:, c, :, :], in_=xT[:, c, :, :])
            wq_sb = sing.tile([P, DC2, 2, fdim], wdt)
            nc.gpsimd.dma_start(out=wq_sb, in_=wq[:, :, :, :])
            wk_sb = sing.tile([P, DC2, 2, fdim], wdt)
            nc.gpsimd.dma_start(out=wk_sb, in_=wk[:, :, :, :])
            wp_sb = sing.tile([P, FC, d], bf16)
            nc.gpsimd.dma_start(out=wp_sb, in_=wp[:, :, :])
            bias_sb = sing.tile([P, d], f32)
            nc.gpsimd.dma_start(out=bias_sb, in_=bias[:, :])

            qT_sb = sing.tile([P, NP, tok], bf16)
            kT_sb = sing.tile([P, NP, tok], bf16)
            vaug_sb = sing.tile([P, KC, h_loc, hd + 1], bf16)
            # ones column = WS so att*WS / (sums*WS) cancels the v scale
            nc.gpsimd.memset(vaug_sb, WS if USE_FP8_PROJ else 1.0)
            aoT_sb = sing.tile([P, NP, tok], bf16)

            def proj_mm(ps, lhsT_sb, lhs_cols, rhs_sb, rhs_cols, ps_cols):
                """accumulate over the folded d contraction into ps[:, ps_cols]"""
                for c in range(DC2):
                    if USE_FP8_PROJ:
                        nc.tensor.matmul(
                            ps[:, ps_cols[0], ps_cols[1]],
                            lhsT_sb[:, c, :, lhs_cols],
                            rhs_sb[:, c, :, rhs_cols],
                            start=(c == 0),
                            stop=(c == DC2 - 1),
                            perf_mode=DR,
                        )
                    else:
                        for i in range(2):
                            nc.tensor.matmul(
                                ps[:, ps_cols[0], ps_cols[1]],
                                lhsT_sb[:, c, i, lhs_cols],
                                rhs_sb[:, c, i, rhs_cols],
                                start=(c == 0 and i == 0),
                                stop=(c == DC2 - 1 and i == 1),
                            )

            # ---- v (token-major) into vaug ------------------------------------
            for tb2 in range(TB // 2):
                ps = psA.tile([P, 2, MMN], f32, tag="ps")
                for half in range(2):
                    tb = 2 * tb2 + half
                    proj_mm(ps, xT_sb, slice(tb * P, (tb + 1) * P),
                            wv_sb, slice(0, fdim), (half, slice(0, fdim)))
                nc.scalar.copy(
                    vaug_sb[:, 2 * tb2:2 * tb2 + 2, :, 0:hd],
                    ps.rearrange("p e (h f) -> p e h f", h=h_loc),
                )

            # ---- q/k projections ----------------------------------------------
            for p in range(NP):
                for w_sb, dst in ((wq_sb, qT_sb), (wk_sb, kT_sb)):
                    pst = [psA.tile([P, 2, MMN], f32, tag="ps", name=f"pst{i2}")
                           for i2 in range(2)]
                    for c in range(DC2):
                        if USE_FP8_PROJ:
                            for n4 in range(4):
                                nc.tensor.matmul(
                                    pst[n4 // 2][:, n4 % 2, :],
                                    w_sb[:, c, :, p * P:(p + 1) * P],
                                    xT_sb[:, c, :, n4 * MMN:(n4 + 1) * MMN],
                                    start=(c == 0),
                                    stop=(c == DC2 - 1),
                                    perf_mode=DR,
                                )
                        else:
                            for i in range(2):
                                for n4 in range(4):
                                    nc.tensor.matmul(
                                        pst[n4 // 2][:, n4 % 2, :],
                                        w_sb[:, c, i, p * P:(p + 1) * P],
                                        xT_sb[:, c, i, n4 * MMN:(n4 + 1) * MMN],
                                        start=(c == 0 and i == 0),
                                        stop=(c == DC2 - 1 and i == 1),
                                    )
                    for h2 in range(2):
                        nc.scalar.copy(
                            dst[:, p, h2 * 2 * MMN:(h2 + 1) * 2 * MMN], pst[h2])

            # ---- attention (qh outer; proj per qh block) -----------------------
            def proj_block(tb):
                pp = psA.tile([P, 2, MMN], f32, tag="ps", name="pp")
                for fc in range(FC):
                    for m0 in range(2):
                        nc.tensor.matmul(
                            pp[:, m0, :],
                            aoT_sb[:, fc, tb * P:(tb + 1) * P],
                            wp_sb[:, fc, m0 * MMN:(m0 + 1) * MMN],
                            start=(fc == 0),
                            stop=(fc == FC - 1),
                        )
                ot = outp.tile([P, d], f32, tag="ot", name="ot")
                nc.vector.tensor_add(ot, pp.rearrange("p e m -> p (e m)"), bias_sb)
                nc.sync.dma_start(out=out[tb * P:(tb + 1) * P, :], in_=ot)

            for qh in range(QH):
                q0 = qh * QT
                qsl = slice(q0, q0 + QT)
                for p in range(NP):
                    po = psO.tile([hd + 1, 2, QT], f32, tag="po", name="po")
                    ats = {}
                    for g in range(KC // 2):
                        pg = [psA.tile([P, 2, QT], f32, tag="ps", name=f"pg{i2}")
                              for i2 in range(2)]
                        # scores: e0/e1 adjacent -> concurrent 64-row tiles
                        for half in range(2):
                            kc = 2 * g + half
                            for e in range(2):
                                esl = slice(e * hd, (e + 1) * hd)
                                nc.tensor.matmul(
                                    pg[half][:, e, :],
                                    kT_sb[esl, p, kc * P:(kc + 1) * P],
                                    qT_sb[esl, p, qsl],
                                    start=True,
                                    stop=True,
                                )
                        # exp: even kc on ScalarE (exact), odd on DVE (bit-trick)
                        for half in range(2):
                            kc = 2 * g + half
                            at = atp.tile([P, 2, QT], bf16, tag="at")
                            if DVE_EXP and half == 1:
                                nc.vector.tensor_scalar(
                                    at.bitcast(u16), pg[half],
                                    A_sch, B_sch, Mult, Add)
                            else:
                                nc.scalar.activation(at, pg[half], Exp,
                                                     scale=scale_eff)
                            ats[kc] = at
                        # AV: accumulate po over kc
                        for half in range(2):
                            kc = 2 * g + half
                            for e in range(2):
                                nc.tensor.matmul(
                                    po[:, e, :],
                                    vaug_sb[:, kc, 2 * p + e, :],
                                    ats[kc][:, e, :],
                                    start=(kc == 0),
                                    stop=(kc == KC - 1),
                                )
                    # normalize off the critical path
                    stg = work.tile([hd + 1, 2, QT], f32, tag="stg")
                    nc.scalar.copy(stg, po)
                    sums_dr = dscr.tile([1, 2, QT], f32, tag="sums_dr")
                    nc.sync.dma_start(out=sums_dr, in_=stg[hd:hd + 1, :, :])
                    rsrc = work.tile([hd, 2, QT], f32, tag="rsrc")
                    nc.sync.dma_start(out=rsrc, in_=sums_dr.partition_broadcast(hd))
                    rec = work.tile([hd, 2, QT], f32, tag="rec")
                    nc.vector.reciprocal_approx_fast(rec, rsrc)
                    for e in range(2):
                        nc.gpsimd.tensor_mul(
                            aoT_sb[e * hd:(e + 1) * hd, p, qsl],
                            stg[0:hd, e, :],
                            rec[:, e, :],
                        )
                for tb in range(qh * (QT // P), (qh + 1) * (QT // P)):
                    proj_block(tb)

    nc.compile()
    return nc


def get_program(tok=N, d=D, h_loc=HLOC, hd=HD, debug=False, repeat=1):
    key = (tok, d, h_loc, hd, debug, repeat)
    if key not in _PROG_CACHE:
        _PROG_CACHE[key] = _build_program(tok, d, h_loc, hd, debug=debug,
                                          repeat=repeat)
    return _PROG_CACHE[key]


def _fold_d(a, np_dtype):
    """[d, m] -> [128, d//256, 2, m] with d_row = c2*256 + i*128 + p."""
    dd, m = a.shape
    return np.ascontiguousarray(
        a.reshape(dd // 256, 2, 128, m).transpose(2, 0, 1, 3)).astype(np_dtype)


def make_in_maps(inputs_f32, W_qkv, W_proj, b_proj):
    """Shard full inputs into the 8 per-core input dicts."""
    import ml_dtypes

    bf16 = ml_dtypes.bfloat16
    f8 = ml_dtypes.float8_e4m3fn if USE_FP8_PROJ else bf16
    ws = WS if USE_FP8_PROJ else 1.0
    in_maps = []
    for core in range(NCORES):
        b_idx, j = divmod(core, TP)
        f0, f1 = j * FDIM, (j + 1) * FDIM
        xT = _fold_d(np.ascontiguousarray(inputs_f32[b_idx].T), f8)
        wq_s = _fold_d(W_qkv[:, f0:f1] * ws, f8)
        wk_s = _fold_d(W_qkv[:, D + f0:D + f1] * ws, f8)
        wv_s = _fold_d(W_qkv[:, 2 * D + f0:2 * D + f1] * ws, f8)
        wp_s = np.ascontiguousarray(
            W_proj[f0:f1, :].reshape(FDIM // 128, 128, D).transpose(1, 0, 2)
        ).astype(bf16)
        if j == 0:
            bias_rep = np.broadcast_to(b_proj.astype(np.float32), (128, D)).copy()
        else:
            bias_rep = np.zeros((128, D), np.float32)
        in_maps.append(
            {"xT": xT, "wq": wq_s, "wk": wk_s, "wv": wv_s, "wp": wp_s,
             "bias": bias_rep}
        )
    return in_maps


def kernel(inputs, W_qkv, W_proj, b_proj):
    from concourse.bass_utils import run_bass_kernel_spmd

    inputs = np.asarray(inputs, dtype=np.float32)
    W_qkv = np.asarray(W_qkv, dtype=np.float32)
    W_proj = np.asarray(W_proj, dtype=np.float32)
    b_proj = np.asarray(b_proj, dtype=np.float32)

    nc = get_program()
    in_maps = make_in_maps(inputs, W_qkv, W_proj, b_proj)
    res = run_bass_kernel_spmd(nc, in_maps, core_ids=list(range(NCORES)))
    outs = [r["out"].astype(np.float32) for r in res.results]
    full = np.stack([outs[TP * b] + outs[TP * b + 1] for b in range(B)], axis=0)
    return full


# revision 19
# speedup vs baseline: 1.0582x; 1.0566x over previous
"""Multi-head attention (B=4, N=2048, D=1024, H=16) on 8 TRN2 NeuronCores.

Sharding: DP=4 over batch x TP=2 over heads (megatron style).
  core c = 2*batch + j   (j in {0,1} = head-group half)

Per-core pipeline (vs the bf16 baseline, ~20% faster):
  - QK^T (K=64) issues the two head-halves e=0/e=1 back-to-back with
    tile_position (0,0)/(64,0): 64x128-row-tiled matmuls whose LDWEIGHTS
    hide under the other tile's matmul (measured 254ns/MM vs 323 serial).
  - all 16 score chunks of a (head-pair, query-tile) are computed+exp'd as
    one batch (staying in 64-row mode), then the AV matmuls run as one
    batch: a single 64<->128 tiling-mode switch per head-tile instead of
    one per chunk (mode switches measured ~240-800ns each).
  - softmax exp is split 9:7 between ScalarE (exact activation) and
    VectorE (Schraudolph bit-trick: bf16 bits = trunc(S*A + B) by one
    tensor_scalar into a uint16 view, ~3.3% max per-weight err that
    largely cancels in softmax normalization) so evacuation keeps pace
    with the PE; consecutive same-engine exps are avoided (they stall the
    3-deep score PSUM pool).
  - per-kc score pairs share one 2-bank PSUM tile [128, 2, 512]; one exp
    instruction covers both halves (amortizes the 352-cycle ACT overhead).
  - proj for query-tile qh is deferred until after qh+1's attention so the
    normalization chain (sums row bounced via DRAM partition-broadcast,
    reciprocal_approx_fast on DVE, scale on GpSimd) hides off-path.
  - fp8e4+DoubleRow projections were tried and rejected: e4m3 quantization
    of x/W_qkv alone costs ~2-3e-2 rel err (gate is 2e-2).

Cost model measured on this stack (hw For_i-loop slope timing): every
matmul pays LDWEIGHTS serially (cost = N_free/2.4GHz + M_cols/1.2GHz);
the toolchain emits one LDWEIGHTS per matmul unconditionally.
"""

import numpy as np

B, N, D, H = 4, 2048, 1024, 16
HD = 64
NCORES = 8
TP = 2
HLOC = H // TP          # 8 heads per core
FDIM = HLOC * HD        # 512

WS = 64.0               # 2^6 host-side scale on W_qkv (fp8 denormal escape)
USE_FP8_PROJ = False    # fp8e4+DoubleRow for the q/k/v projections (too lossy: ~3e-2)
DVE_EXP = True          # odd kc chunks use the DVE bit-trick exp
SCH_C = 5.6             # Schraudolph constant (min-max rel err ~3.3%)

_PROG_CACHE = {}


def _build_program(tok, d, h_loc, hd, debug=False, repeat=1, hw_iters=0):
    """Build the single-core Bass/Tile program (same program runs SPMD on all cores).

    hw_iters>0 wraps the body in a hardware For_i loop (constant NEFF size
    regardless of trip count — used for honest device-time measurement)."""
    import concourse.tile as tile
    from concourse import bacc, mybir

    f32 = mybir.dt.float32
    bf16 = mybir.dt.bfloat16
    f8 = mybir.dt.float8e4
    u16 = mybir.dt.uint16
    Exp = mybir.ActivationFunctionType.Exp
    Mult = mybir.AluOpType.mult
    Add = mybir.AluOpType.add
    DR = mybir.MatmulPerfMode.DoubleRow

    P = 128
    DC2 = d // 256              # folded contraction chunks (4): 256 d-rows each
    NP = h_loc // 2             # head pairs (4)
    TB = tok // P               # token blocks (16)
    KC = tok // P               # key chunks (16)
    fdim = h_loc * hd           # local feature dim (512)
    FC = fdim // P              # proj contraction chunks (4)
    QT = 512                    # query tile (1 PSUM bank)
    QH = tok // QT              # query tiles (4)
    MMN = 512
    scale_eff = float(hd) ** -0.5 / (WS * WS) if USE_FP8_PROJ else float(hd) ** -0.5
    log2e = 1.4426950408889634
    A_sch = 128.0 * log2e * scale_eff
    B_sch = 127.0 * 128.0 - SCH_C + 0.5

    wdt = f8 if USE_FP8_PROJ else bf16

    nc = bacc.Bacc("TRN2", target_bir_lowering=False, debug=debug)

    # host-folded layouts: [p, c2, i, *] with d_row = c2*256 + i*128 + p
    xT = nc.dram_tensor("xT", [P, DC2, 2, tok], wdt, kind="ExternalInput")
    wq = nc.dram_tensor("wq", [P, DC2, 2, fdim], wdt, kind="ExternalInput")
    wk = nc.dram_tensor("wk", [P, DC2, 2, fdim], wdt, kind="ExternalInput")
    wv = nc.dram_tensor("wv", [P, DC2, 2, fdim], wdt, kind="ExternalInput")
    wp = nc.dram_tensor("wp", [P, FC, d], bf16, kind="ExternalInput")
    bias = nc.dram_tensor("bias", [P, d], f32, kind="ExternalInput")
    out = nc.dram_tensor("out", [tok, d], f32, kind="ExternalOutput")

    with tile.TileContext(nc) as tc:
        with (
            tc.tile_pool(name="sing", bufs=1) as sing,
            tc.tile_pool(name="psA", bufs=3, space="PSUM") as psA,
            tc.tile_pool(name="psO", bufs=1, space="PSUM") as psO,
            tc.tile_pool(name="atp", bufs=17) as atp,
            tc.tile_pool(name="work", bufs=2) as work,
            tc.tile_pool(name="outp", bufs=2) as outp,
            tc.tile_pool(name="dscr", bufs=8, space="DRAM") as dscr,
        ):
          def body():
            # ---- resident loads (first v chunk lands ASAP) ---------------------
            wv_sb = sing.tile([P, DC2, 2, fdim], wdt)
            xT_sb = sing.tile([P, DC2, 2, tok], wdt)
            xq = (nc.sync, nc.scalar, nc.sync, nc.scalar)
            wq_ = (nc.gpsimd, nc.gpsimd, nc.gpsimd, nc.gpsimd)
            nc.gpsimd.dma_start(out=wv_sb[:, 0, :, :], in_=wv[:, 0, :, :])
            nc.sync.dma_start(out=xT_sb[:, 0, :, :], in_=xT[:, 0, :, :])
            for c in range(1, DC2):
                wq_[c].dma_start(out=wv_sb[:, c, :, :], in_=wv[:, c, :, :])
                xq[c].dma_start(out=xT_sb[:, c, :, :], in_=xT[:, c, :, :])
            wq_sb = sing.tile([P, DC2, 2, fdim], wdt)
            nc.gpsimd.dma_start(out=wq_sb, in_=wq[:, :, :, :])
            wk_sb = sing.tile([P, DC2, 2, fdim], wdt)
            nc.gpsimd.dma_start(out=wk_sb, in_=wk[:, :, :, :])
            wp_sb = sing.tile([P, FC, d], bf16)
            nc.gpsimd.dma_start(out=wp_sb, in_=wp[:, :, :])
            bias_sb = sing.tile([P, d], f32)
            nc.gpsimd.dma_start(out=bias_sb, in_=bias[:, :])

            qT_sb = sing.tile([P, NP, tok], bf16)
            kT_sb = sing.tile([P, NP, tok], bf16)
            vaug_sb = sing.tile([P, KC, h_loc, hd + 1], bf16)
            # ones column = WS so att*WS / (sums*WS) cancels the v scale
            nc.gpsimd.memset(vaug_sb, WS if USE_FP8_PROJ else 1.0)
            aoT_sb = sing.tile([P, NP, tok], bf16)

            def proj_mm(ps, lhsT_sb, lhs_cols, rhs_sb, rhs_cols, ps_cols):
                """accumulate over the folded d contraction into ps[:, ps_cols]"""
                for c in range(DC2):
                    if USE_FP8_PROJ:
                        nc.tensor.matmul(
                            ps[:, ps_cols[0], ps_cols[1]],
                            lhsT_sb[:, c, :, lhs_cols],
                            rhs_sb[:, c, :, rhs_cols],
                            start=(c == 0),
                            stop=(c == DC2 - 1),
                            perf_mode=DR,
                        )
                    else:
                        for i in range(2):
                            nc.tensor.matmul(
                                ps[:, ps_cols[0], ps_cols[1]],
                                lhsT_sb[:, c, i, lhs_cols],
                                rhs_sb[:, c, i, rhs_cols],
                                start=(c == 0 and i == 0),
                                stop=(c == DC2 - 1 and i == 1),
                            )

            # ---- v (token-major) into vaug ------------------------------------
            for tb2 in range(TB // 2):
                ps = psA.tile([P, 2, MMN], f32, tag="ps")
                for half in range(2):
                    tb = 2 * tb2 + half
                    proj_mm(ps, xT_sb, slice(tb * P, (tb + 1) * P),
                            wv_sb, slice(0, fdim), (half, slice(0, fdim)))
                nc.scalar.copy(
                    vaug_sb[:, 2 * tb2:2 * tb2 + 2, :, 0:hd],
                    ps.rearrange("p e (h f) -> p e h f", h=h_loc),
                )

            # ---- q/k projections ----------------------------------------------
            for p in range(NP):
                for w_sb, dst in ((wq_sb, qT_sb), (wk_sb, kT_sb)):
                    pst = [psA.tile([P, 2, MMN], f32, tag="ps", name=f"pst{i2}")
                           for i2 in range(2)]
                    for c in range(DC2):
                        if USE_FP8_PROJ:
                            for n4 in range(4):
                                nc.tensor.matmul(
                                    pst[n4 // 2][:, n4 % 2, :],
                                    w_sb[:, c, :, p * P:(p + 1) * P],
                                    xT_sb[:, c, :, n4 * MMN:(n4 + 1) * MMN],
                                    start=(c == 0),
                                    stop=(c == DC2 - 1),
                                    perf_mode=DR,
                                )
                        else:
                            for i in range(2):
                                for n4 in range(4):
                                    nc.tensor.matmul(
                                        pst[n4 // 2][:, n4 % 2, :],
                                        w_sb[:, c, i, p * P:(p + 1) * P],
                                        xT_sb[:, c, i, n4 * MMN:(n4 + 1) * MMN],
                                        start=(c == 0 and i == 0),
                                        stop=(c == DC2 - 1 and i == 1),
                                    )
                    for h2 in range(2):
                        nc.scalar.copy(
                            dst[:, p, h2 * 2 * MMN:(h2 + 1) * 2 * MMN], pst[h2])

            # ---- attention (qh outer; proj per qh block) -----------------------
            def proj_block(tb):
                pp = psA.tile([P, 2, MMN], f32, tag="ps", name="pp")
                for fc in range(FC):
                    for m0 in range(2):
                        nc.tensor.matmul(
                            pp[:, m0, :],
                            aoT_sb[:, fc, tb * P:(tb + 1) * P],
                            wp_sb[:, fc, m0 * MMN:(m0 + 1) * MMN],
                            start=(fc == 0),
                            stop=(fc == FC - 1),
                        )
                ot = outp.tile([P, d], f32, tag="ot", name="ot")
                nc.vector.tensor_add(ot, pp.rearrange("p e m -> p (e m)"), bias_sb)
                nc.sync.dma_start(out=out[tb * P:(tb + 1) * P, :], in_=ot)

            for qh in range(QH):
                q0 = qh * QT
                qsl = slice(q0, q0 + QT)
                for p in range(NP):
                    po = psO.tile([hd + 1, 2, QT], f32, tag="po", name="po")
                    ats = []
                    # scores + exp batch: stays in 64-row mode throughout;
                    # e0/e1 adjacent -> concurrent row tiles
                    for kc in range(KC):
                        pg = psA.tile([P, 2, QT], f32, tag="ps", name="pg")
                        for e in range(2):
                            esl = slice(e * hd, (e + 1) * hd)
                            nc.tensor.matmul(
                                pg[:, e, :],
                                kT_sb[esl, p, kc * P:(kc + 1) * P],
                                qT_sb[esl, p, qsl],
                                start=True,
                                stop=True,
                            )
                        at = atp.tile([P, 2, QT], bf16, tag="at")
                        if DVE_EXP and kc in (1, 3, 5, 7, 9, 11, 13):
                            nc.vector.tensor_scalar(
                                at.bitcast(u16), pg, A_sch, B_sch, Mult, Add)
                        else:
                            nc.scalar.activation(at, pg, Exp, scale=scale_eff)
                        ats.append(at)
                    # AV batch: single 64->128 mode switch per (p, qh)
                    for e in range(2):
                        for kc in range(KC):
                            nc.tensor.matmul(
                                po[:, e, :],
                                vaug_sb[:, kc, 2 * p + e, :],
                                ats[kc][:, e, :],
                                start=(kc == 0),
                                stop=(kc == KC - 1),
                            )
                    # normalize off the critical path
                    stg = work.tile([hd + 1, 2, QT], f32, tag="stg")
                    nc.scalar.copy(stg, po)
                    sums_dr = dscr.tile([1, 2, QT], f32, tag="sums_dr")
                    nc.sync.dma_start(out=sums_dr, in_=stg[hd:hd + 1, :, :])
                    rsrc = work.tile([hd, 2, QT], f32, tag="rsrc")
                    nc.sync.dma_start(out=rsrc, in_=sums_dr.partition_broadcast(hd))
                    rec = work.tile([hd, 2, QT], f32, tag="rec")
                    nc.vector.reciprocal_approx_fast(rec, rsrc)
                    for e in range(2):
                        nc.gpsimd.tensor_mul(
                            aoT_sb[e * hd:(e + 1) * hd, p, qsl],
                            stg[0:hd, e, :],
                            rec[:, e, :],
                        )
                # proj deferred one qh so the normalize DMA-bounce latency
                # hides under the next qh's attention
                if qh > 0:
                    for tb in range((qh - 1) * (QT // P), qh * (QT // P)):
                        proj_block(tb)
            for tb in range((QH - 1) * (QT // P), QH * (QT // P)):
                proj_block(tb)

          if hw_iters:
              with tc.For_i(0, hw_iters, 1):
                  body()
          else:
              for _rep in range(repeat):
                  body()

    nc.compile()
    return nc


def get_program(tok=N, d=D, h_loc=HLOC, hd=HD, debug=False, repeat=1,
                hw_iters=0):
    key = (tok, d, h_loc, hd, debug, repeat, hw_iters)
    if key not in _PROG_CACHE:
        _PROG_CACHE[key] = _build_program(tok, d, h_loc, hd, debug=debug,
                                          repeat=repeat, hw_iters=hw_iters)
    return _PROG_CACHE[key]


def _fold_d(a, np_dtype):
    """[d, m] -> [128, d//256, 2, m] with d_row = c2*256 + i*128 + p."""
    dd, m = a.shape
    return np.ascontiguousarray(
        a.reshape(dd // 256, 2, 128, m).transpose(2, 0, 1, 3)).astype(np_dtype)


def make_in_maps(inputs_f32, W_qkv, W_proj, b_proj):
    """Shard full inputs into the 8 per-core input dicts."""
    import ml_dtypes

    bf16 = ml_dtypes.bfloat16
    f8 = ml_dtypes.float8_e4m3fn if USE_FP8_PROJ else bf16
    ws = WS if USE_FP8_PROJ else 1.0
    in_maps = []
    for core in range(NCORES):
        b_idx, j = divmod(core, TP)
        f0, f1 = j * FDIM, (j + 1) * FDIM
        xT = _fold_d(np.ascontiguousarray(inputs_f32[b_idx].T), f8)
        wq_s = _fold_d(W_qkv[:, f0:f1] * ws, f8)
        wk_s = _fold_d(W_qkv[:, D + f0:D + f1] * ws, f8)
        wv_s = _fold_d(W_qkv[:, 2 * D + f0:2 * D + f1] * ws, f8)
        wp_s = np.ascontiguousarray(
            W_proj[f0:f1, :].reshape(FDIM // 128, 128, D).transpose(1, 0, 2)
        ).astype(bf16)
        if j == 0:
            bias_rep = np.broadcast_to(b_proj.astype(np.float32), (128, D)).copy()
        else:
            bias_rep = np.zeros((128, D), np.float32)
        in_maps.append(
            {"xT": xT, "wq": wq_s, "wk": wk_s, "wv": wv_s, "wp": wp_s,
             "bias": bias_rep}
        )
    return in_maps


def kernel(inputs, W_qkv, W_proj, b_proj):
    from concourse.bass_utils import run_bass_kernel_spmd

    inputs = np.asarray(inputs, dtype=np.float32)
    W_qkv = np.asarray(W_qkv, dtype=np.float32)
    W_proj = np.asarray(W_proj, dtype=np.float32)
    b_proj = np.asarray(b_proj, dtype=np.float32)

    nc = get_program()
    in_maps = make_in_maps(inputs, W_qkv, W_proj, b_proj)
    res = run_bass_kernel_spmd(nc, in_maps, core_ids=list(range(NCORES)))
    outs = [r["out"].astype(np.float32) for r in res.results]
    full = np.stack([outs[TP * b] + outs[TP * b + 1] for b in range(B)], axis=0)
    return full
